# revision 29
# baseline (speedup 1.0000x reference)
"""Trainium2 Bass kernel for nn_AttentiveResidualDenseBlock_5C.

Strategy (pure data parallelism, 16 samples / 8 cores = 2 per core):
  - Activations live in SBUF in a zero-padded [98x98] spatial layout (bf16),
    organized as 3 channel-tiles of 128 partitions per sample:
      tile0: ch 0-63 = x (parts 0-63), ch 64-127 = x1 (parts 64-127)
      tile1: x2 (parts 0-63), x3 (parts 64-127)
      tile2: x4 (parts 0-63)
  - 3x3 conv = 9 shifted matmuls over the flat padded index space; output
    chunks are 4 rows (N=392) accumulated in PSUM over (ctile, offset).
  - CondConv attention: channel sums come free from the evacuation ACT's
    accum_out; tiny PE matmuls do the MLP; softmax on [1,8].
  - Per-sample kernel bank aggregation: bf16 broadcast-multiply (DVE) +
    grouped tensor_reduce over k (innermost 8) + ACT cast to bf16.
  - Two samples are ping-pong pipelined so DVE aggregation + attention of
    one sample hides under the other sample's conv matmuls.
"""
import os
import sys

sys.path.insert(0, "/opt/trn_rl_repo")

import numpy as np
import ml_dtypes

import concourse.bass as bass
import concourse.bacc as bacc
import concourse.mybir as mybir
import concourse.tile as tile
from concourse.bass_utils import run_bass_kernel_spmd
from contextlib import ExitStack

F32 = mybir.dt.float32
BF16 = mybir.dt.bfloat16
AF = mybir.ActivationFunctionType
ALU = mybir.AluOpType
AX = mybir.AxisListType

NF, GC, K, B, H, W = 64, 64, 8, 16, 96, 96
CINS = [64, 128, 192, 256, 320]
HIDS = [c // 4 for c in CINS]          # 16,32,48,64,80
NTILES = [1, 1, 2, 2, 3]
NCORES = 8
NLOC = B // NCORES                      # 2 samples per core
PW = 98                                 # padded row length
GRID = PW * PW                          # 9604
MARG = 4                                # left margin columns
RMARG = 300                             # right margin (room for strided views)
XCOLS = MARG + GRID + RMARG
NCH = 24                                # conv output chunks (4 rows each)
CHROWS = 4
CHN = CHROWS * PW                       # 392 matmul N
HW = H * W                              # 9216
L1OFF = [0, 16, 80, 224, 480]           # aw1t col offsets per layer

# destination of layer i output (0-based layer): (ctile, base_partition)
DEST = [(0, 64), (1, 0), (1, 64), (2, 0), None]

LAST_EXEC_NS = None
LAST_TRACE = None

_cache = {}


def _build(has_bias: bool):
    nc = bacc.Bacc("TRN2", target_bir_lowering=False, debug=False)

    x_d = nc.dram_tensor("x", [NLOC, NF, HW], F32, kind="ExternalInput")
    wb_d = [nc.dram_tensor(f"wb{i+1}", [CINS[i], 4608], BF16, kind="ExternalInput")
            for i in range(5)]
    wb5p_d = nc.dram_tensor("wb5p", [128, 3072], BF16, kind="ExternalInput")
    aw1t_d = nc.dram_tensor("aw1t", [128, 880], F32, kind="ExternalInput")
    aw2t_d = nc.dram_tensor("aw2t", [80, 40], F32, kind="ExternalInput")
    ab1p_d = nc.dram_tensor("ab1p", [128, 8], F32, kind="ExternalInput")
    ab2p_d = nc.dram_tensor("ab2p", [1, 40], F32, kind="ExternalInput")
    bmat_d = nc.dram_tensor("bmat", [8, 320], F32, kind="ExternalInput")
    ones_d = nc.dram_tensor("ones_", [1, 128], F32, kind="ExternalInput")
    out_d = nc.dram_tensor("out", [NLOC, NF, HW], F32, kind="ExternalOutput")

    with tile.TileContext(nc) as tc, ExitStack() as ctx:
        const = ctx.enter_context(tc.tile_pool(name="const", bufs=1))
        xbp = ctx.enter_context(tc.tile_pool(name="xbp", bufs=1))
        wbs = ctx.enter_context(tc.tile_pool(name="wbs", bufs=2))
        aggp = ctx.enter_context(tc.tile_pool(name="aggp", bufs=2))
        stg = ctx.enter_context(tc.tile_pool(name="stg", bufs=2))
        attp = ctx.enter_context(tc.tile_pool(name="attp", bufs=2))
        pconv = ctx.enter_context(tc.tile_pool(name="pconv", bufs=2, space="PSUM"))
        psml = ctx.enter_context(tc.tile_pool(name="psml", bufs=2, space="PSUM"))

        # ---- constants
        aw1t = const.tile([128, 880], F32, name="aw1t_s")
        nc.sync.dma_start(aw1t[:], aw1t_d[:, :])
        aw2t = const.tile([80, 40], F32, name="aw2t_s")
        nc.sync.dma_start(aw2t[:], aw2t_d[:, :])
        ab1p = const.tile([128, 8], F32, name="ab1p_s")
        nc.sync.dma_start(ab1p[:], ab1p_d[:, :])
        ab2p = const.tile([1, 40], F32, name="ab2p_s")
        nc.sync.dma_start(ab2p[:], ab2p_d[:, :])
        ones = const.tile([1, 128], F32, name="ones_s")
        nc.sync.dma_start(ones[:], ones_d[:, :])
        if has_bias:
            bmat = const.tile([8, 320], F32, name="bmat_s")
            nc.sync.dma_start(bmat[:], bmat_d[:, :])

        # ---- persistent per-sample state
        xb = [[xbp.tile([128, XCOLS], BF16, name=f"xb{s}_{t}") for t in range(3)]
              for s in range(NLOC)]
        wagg = [xbp.tile([128, 3 * 576], BF16, name=f"wagg{s}") for s in range(NLOC)]
        attn_bc = [xbp.tile([128, 8], BF16, name=f"attnbc{s}") for s in range(NLOC)]
        pooled = [xbp.tile([128, 8], F32, name=f"pooled{s}") for s in range(NLOC)]
        accs = [xbp.tile([128, NCH], F32, name=f"accs{s}") for s in range(NLOC)]
        bk = [xbp.tile([128, 8], F32, name=f"bk{s}") for s in range(NLOC)]
        if not has_bias:
            for s in range(NLOC):
                nc.vector.memset(bk[s][:], 0.0)
        for s in range(NLOC):
            for t in range(3):
                nc.vector.memset(xb[s][t][:, 0:MARG], 0.0)
                nc.vector.memset(xb[s][t][:, MARG + GRID:XCOLS], 0.0)

        def grid_view(s, t, lo):
            return (xb[s][t][lo:lo + 64, MARG:MARG + GRID]
                    .rearrange("p (r c) -> p r c", c=PW))

        def ring_zero(s, t, lo):
            v = grid_view(s, t, lo)
            nc.vector.memset(v[:, 0:PW:97, :], 0.0)
            nc.vector.memset(v[:, :, 0:PW:97], 0.0)

        def load_x(s):
            nc.vector.memset(pooled[s][64:128, 0:1], 0.0)
            for r in range(12):
                xs = stg.tile([64, 8 * 96], F32, name="xs", tag="xs")
                nc.sync.dma_start(xs[:], x_d[s, :, r * 768:(r + 1) * 768])
                dst = grid_view(s, 0, 0)[:, 1 + 8 * r:9 + 8 * r, 1:97]
                src = xs[:, :].rearrange("p (r c) -> p r c", c=96)
                nc.scalar.activation(dst, src, AF.Copy,
                                     accum_out=accs[s][0:64, r:r + 1])
            ring_zero(s, 0, 0)
            nc.vector.tensor_reduce(pooled[s][0:64, 0:1], accs[s][0:64, 0:12],
                                    axis=AX.X, op=ALU.add)

        def attention(s, i):
            """Compute attn_bc[s] (and bk) for layer i from pooled sums."""
            hid = HIDS[i]
            ph = psml.tile([128, 8], F32, name="ph", tag="small")
            for g in range(i + 1):
                off = L1OFF[i] + g * hid
                nc.tensor.matmul(ph[0:hid, 0:1], aw1t[0:64, off:off + hid],
                                 pooled[s][0:64, g:g + 1],
                                 start=(g == 0), stop=False)
                nc.tensor.matmul(ph[0:hid, 0:1], aw1t[64:128, off:off + hid],
                                 pooled[s][64:128, g:g + 1],
                                 start=False, stop=(g == i))
            h_sb = attp.tile([80, 1], F32, name="h_sb", tag="h")
            nc.scalar.activation(h_sb[0:hid, 0:1], ph[0:hid, 0:1], AF.Relu,
                                 bias=ab1p[0:hid, i:i + 1], scale=1.0 / HW)
            pl = psml.tile([128, 8], F32, name="pl", tag="small")
            nc.tensor.matmul(pl[0:1, 0:8], h_sb[0:hid, 0:1],
                             aw2t[0:hid, 8 * i:8 * i + 8], start=True, stop=False)
            nc.tensor.matmul(pl[0:1, 0:8], ones[0:1, 0:1],
                             ab2p[0:1, 8 * i:8 * i + 8], start=False, stop=True)
            e_sb = attp.tile([1, 8], F32, name="e_sb", tag="e")
            ssum = attp.tile([1, 1], F32, name="ssum", tag="ss")
            nc.scalar.activation(e_sb[:], pl[0:1, 0:8], AF.Exp,
                                 accum_out=ssum[0:1, 0:1])
            rinv = attp.tile([1, 1], F32, name="rinv", tag="ri")
            nc.vector.reciprocal(rinv[0:1, 0:1], ssum[0:1, 0:1])
            attf = attp.tile([1, 8], F32, name="attf", tag="af")
            nc.vector.tensor_scalar_mul(attf[0:1, 0:8], e_sb[0:1, 0:8],
                                        rinv[0:1, 0:1])
            pbc = psml.tile([128, 8], F32, name="pbc", tag="small")
            nc.tensor.matmul(pbc[0:128, 0:8], ones[0:1, 0:128], attf[0:1, 0:8],
                             start=True, stop=True)
            nc.scalar.activation(attn_bc[s][:], pbc[0:128, 0:8], AF.Copy)
            if has_bias:
                pat = psml.tile([128, 8], F32, name="pat", tag="small")
                nc.tensor.matmul(pat[0:8, 0:1], attf[0:1, 0:8], ones[0:1, 0:1],
                                 start=True, stop=True)
                at_sb = attp.tile([8, 1], F32, name="at_sb", tag="at")
                nc.scalar.activation(at_sb[:], pat[0:8, 0:1], AF.Copy)
                pbk = psml.tile([128, 8], F32, name="pbk", tag="small")
                nc.tensor.matmul(pbk[0:64, 0:1], bmat[0:8, 64 * i:64 * i + 64],
                                 at_sb[0:8, 0:1], start=True, stop=True)
                sc = 0.2 if i == 4 else 1.0
                nc.scalar.activation(bk[s][0:64, i:i + 1], pbk[0:64, 0:1],
                                     AF.Copy, bias=0.0, scale=sc)
                nc.scalar.activation(bk[s][64:128, i:i + 1], pbk[0:64, 0:1],
                                     AF.Copy, bias=0.0, scale=sc)

        def aggregate(s, i):
            """wagg[s] <- sum_k attn_k * Wb_i[k] for all ctiles of layer i."""
            for t in range(NTILES[i]):
                packed = (i == 4 and t == 2)
                kt = 128 if packed else min(128, CINS[i] - 128 * t)
                ncols = 384 if packed else 576
                wbt = wbs.tile([128, 4608], BF16, name="wbt", tag="wbt")
                if packed:
                    nc.sync.dma_start(wbt[0:128, 0:3072], wb5p_d[:, :])
                else:
                    nc.sync.dma_start(wbt[0:kt, :], wb_d[i][128 * t:128 * t + kt, :])
                tmp = aggp.tile([128, 4608], BF16, name="tmp", tag="tmp")
                wv = wbt[0:kt, 0:8 * ncols].rearrange("p (j k) -> p j k", k=8)
                av = (attn_bc[s][0:kt, 0:8]
                      .rearrange("p (u k) -> p u k", u=1)
                      .broadcast_to((kt, ncols, 8)))
                tv = tmp[0:kt, 0:8 * ncols].rearrange("p (j k) -> p j k", k=8)
                nc.vector.tensor_tensor(tv, wv, av, op=ALU.mult)
                with nc.allow_low_precision("DVE reduce is fp32 internal; bf16 store is a single rounding"):
                    nc.vector.tensor_reduce(wagg[s][0:kt, 576 * t:576 * t + ncols],
                                            tv, axis=AX.X, op=ALU.add)

        def attn_agg(s, i):
            attention(s, i)
            aggregate(s, i)

        def conv(s, i, mid=None):
            """Layer i conv, col-tiled 2x: chunk pair (2q, 2q+1) runs on PE
            column groups 0/1 concurrently (psum partitions 0-63 / 64-127).
            `mid` is emitted after group 0 so its tiny PE ops slot in early
            and its DVE work overlaps the remaining groups."""
            ntl = NTILES[i]
            dest = DEST[i]
            for pg in range(NCH // 6):
                if pg == 1 and mid is not None:
                    mid()
                gg = pconv.tile([128, 3 * 512], F32, name="gg", tag="gg")
                units = []
                for t in range(ntl):
                    if i == 4 and t == 2:
                        for dy in range(3):
                            units.append((wagg[s][0:128, 1152 + 64 * dy:1216 + 64 * dy],
                                          t, 128, PW * (dy - 1) - 1))
                        for dy in range(3):
                            units.append((wagg[s][0:64, 1344 + 64 * dy:1408 + 64 * dy],
                                          t, 64, PW * (dy - 1)))
                    else:
                        kt = min(128, CINS[i] - 128 * t)
                        for off in range(9):
                            units.append((wagg[s][0:kt, 576 * t + 64 * off:576 * t + 64 * off + 64],
                                          t, kt, PW * (off // 3 - 1) + (off % 3 - 1)))
                first = True
                for ui, (wsl, t, kt, dlt) in enumerate(units):
                    if True:
                        last = (ui == len(units) - 1)
                        for j in range(3):
                            q = 3 * pg + j
                            n0A = MARG + PW * (1 + CHROWS * (2 * q))
                            n0B = MARG + PW * (1 + CHROWS * (2 * q + 1))
                            nc.tensor.matmul(gg[0:64, 512 * j:512 * j + CHN], wsl,
                                             xb[s][t][0:kt, n0A + dlt:n0A + dlt + CHN],
                                             start=first, stop=last,
                                             skip_group_check=True)
                            nc.tensor.matmul(gg[64:128, 512 * j:512 * j + CHN], wsl,
                                             xb[s][t][0:kt, n0B + dlt:n0B + dlt + CHN],
                                             start=first, stop=last,
                                             skip_group_check=True)
                        first = False
                for p, pc in ((0, gg), (1, gg)):
                    bp = 64 * p
                    src = (pc[bp:bp + 64, :]
                           .rearrange("p (b x) -> p b x", x=512)[:, :, 0:CHN]
                           .rearrange("p b (r c) -> p b r c", c=PW)[:, :, :, 1:97])
                    if dest is not None:
                        dt, lo = dest
                        base = MARG + PW * (1 + CHROWS * p)
                        dst = (xb[s][dt][lo:lo + 64, base:base + PW * 96]
                               .rearrange("p (g r c) -> p g r c", r=8, c=PW)
                               [:, 3 * pg:3 * pg + 3, 0:CHROWS, 1:97])
                        nc.scalar.activation(dst, src, AF.Prelu,
                                             bias=bk[s][bp:bp + 64, i:i + 1],
                                             scale=1.0, alpha=0.2,
                                             accum_out=accs[s][bp:bp + 64, pg:pg + 1])
                    elif os.environ.get("KERNEL_L5_SIMPLE"):
                        for j in range(3):
                            r = 6 * pg + 2 * j + p
                            srcj = (pc[bp:bp + 64, 512 * j:512 * j + CHN]
                                    .rearrange("p (r c) -> p r c", c=PW)[:, :, 1:97])
                            t5 = stg.tile([64, 384], F32, name="t5", tag="t5")
                            t5v = t5[:, :].rearrange("p (r c) -> p r c", c=96)
                            nc.scalar.activation(t5v, srcj, AF.Identity,
                                                 bias=bk[s][bp:bp + 64, 4:5], scale=0.2)
                            xr = stg.tile([64, 384], F32, name="xr", tag="xr")
                            nc.sync.dma_start(xr[:], x_d[s, :, 384 * r:384 * (r + 1)])
                            oadd = stg.tile([64, 384], F32, name="oadd", tag="oadd")
                            nc.vector.tensor_add(oadd[:], t5[:], xr[:])
                            nc.sync.dma_start(out_d[s, :, 384 * r:384 * (r + 1)], oadd[:])
                    else:
                        t5 = stg.tile([64, 3 * 384], F32, name="t5", tag="t5")
                        t5v = t5[:, :].rearrange("p (b r c) -> p b r c", b=3, c=96)
                        nc.scalar.activation(t5v, src, AF.Identity,
                                             bias=bk[s][bp:bp + 64, 4:5], scale=0.2)
                        xr = stg.tile([64, 3 * 384], F32, name="xr", tag="xr")
                        xv = (x_d[s, :, :].rearrange("p (a b) -> p a b", b=384)
                              [:, 6 * pg + p:6 * pg + p + 5:2, :])
                        nc.sync.dma_start(xr[:, :].rearrange("p (a b) -> p a b", b=384), xv)
                        oadd = stg.tile([64, 3 * 384], F32, name="oadd", tag="oadd")
                        nc.vector.tensor_add(oadd[:], t5[:], xr[:])
                        ov = (out_d[s, :, :].rearrange("p (a b) -> p a b", b=384)
                              [:, 6 * pg + p:6 * pg + p + 5:2, :])
                        nc.sync.dma_start(ov, oadd[:, :].rearrange("p (a b) -> p a b", b=384))

        def post(s, i):
            """Ring zero + pooled sums for layer i output (i < 4)."""
            dt, lo = DEST[i]
            ring_zero(s, dt, lo)
            if i == 3:
                # duplicate x4 shifted left by 2 into tile2's free upper half:
                # enables K=128 packing of offset pairs (dy,0)+(dy,2) in L5
                nc.vector.tensor_copy(xb[s][2][64:128, 0:XCOLS - 2],
                                      xb[s][2][0:64, 2:XCOLS])
            nc.vector.tensor_reduce(pooled[s][0:64, i + 1:i + 2],
                                    accs[s][0:64, 0:4], axis=AX.X, op=ALU.add)
            nc.vector.tensor_reduce(pooled[s][64:128, i + 1:i + 2],
                                    accs[s][64:128, 0:4], axis=AX.X, op=ALU.add)

        # ---------- schedule ----------
        def schedule():
            load_x(0)
            load_x(1)
            attn_agg(0, 0)
            for i in range(5):
                conv(0, i, mid=(lambda i=i: attn_agg(1, i)))
                if i < 4:
                    post(0, i)
                conv(1, i, mid=(lambda i=i: attn_agg(0, i + 1)) if i < 4 else None)
                if i < 4:
                    post(1, i)

        repeat = int(os.environ.get("KERNEL_REPEAT", "1"))
        if repeat > 1:
            with tc.For_i(0, repeat, 1):
                schedule()
        else:
            schedule()

    nc.compile()
    return nc


def _prep_inputs(inputs):
    """Host-side packing of weights; returns common map + per-core x shards."""
    common = {}
    for i in range(5):
        w = np.asarray(inputs[f"w{i+1}"], np.float32)
        cin = CINS[i]
        wb = np.transpose(w, (2, 3, 4, 1, 0)).reshape(cin, 4608)
        common[f"wb{i+1}"] = wb.astype(ml_dtypes.bfloat16)
    # packed bank for L5 tile2 (channels 256-319): 3 packed (dy,0)+(dy,2)
    # K=128 blocks + 3 single (dy,1) K=64 blocks; [128, (6*64) * 8]
    w5 = np.asarray(inputs["w5"], np.float32)      # [8, 64, 320, 3, 3]
    w5t = w5[:, :, 256:320]                        # [8, 64, 64, 3, 3]
    wp = np.zeros((128, 6 * 64, 8), np.float32)
    for dy in range(3):
        # packed block dy: rows 0-63 = (dy,0), rows 64-127 = (dy,2)
        wp[0:64, 64 * dy:64 * dy + 64] = np.transpose(w5t[:, :, :, dy, 0], (2, 1, 0))
        wp[64:128, 64 * dy:64 * dy + 64] = np.transpose(w5t[:, :, :, dy, 2], (2, 1, 0))
        # single block dy: rows 0-63 = (dy,1)
        wp[0:64, 192 + 64 * dy:256 + 64 * dy] = np.transpose(w5t[:, :, :, dy, 1], (2, 1, 0))
    common["wb5p"] = wp.reshape(128, 3072).astype(ml_dtypes.bfloat16)
    aw1t = np.zeros((128, 880), np.float32)
    aw2t = np.zeros((80, 40), np.float32)
    ab1p = np.zeros((128, 8), np.float32)
    ab2p = np.zeros((1, 40), np.float32)
    bmat = np.zeros((8, 320), np.float32)
    for i in range(5):
        hid = HIDS[i]
        aw1 = np.asarray(inputs[f"aw1_{i+1}"], np.float32)
        for g in range(i + 1):
            blk = aw1[:, 64 * g:64 * (g + 1)].T
            aw1t[0:64, L1OFF[i] + g * hid:L1OFF[i] + (g + 1) * hid] = blk
            aw1t[64:128, L1OFF[i] + g * hid:L1OFF[i] + (g + 1) * hid] = blk
        aw2t[0:hid, 8 * i:8 * i + 8] = np.asarray(inputs[f"aw2_{i+1}"], np.float32).T
        ab1p[0:hid, i] = np.asarray(inputs[f"ab1_{i+1}"], np.float32)
        ab2p[0, 8 * i:8 * i + 8] = np.asarray(inputs[f"ab2_{i+1}"], np.float32)
        bmat[:, 64 * i:64 * (i + 1)] = np.asarray(inputs[f"b{i+1}"], np.float32)
    common["aw1t"] = aw1t
    common["aw2t"] = aw2t
    common["ab1p"] = ab1p
    common["ab2p"] = ab2p
    common["bmat"] = bmat
    common["ones_"] = np.ones((1, 128), np.float32)
    has_bias = bool(np.any(bmat))
    x = np.asarray(inputs["x"], np.float32).reshape(B, NF, HW)
    shards = [np.ascontiguousarray(x[c * NLOC:(c + 1) * NLOC]) for c in range(NCORES)]
    return common, shards, None, has_bias


def kernel(**inputs):
    global LAST_EXEC_NS, LAST_TRACE
    common, shards, _unused, has_bias = _prep_inputs(inputs)
    if has_bias not in _cache:
        _cache[has_bias] = _build(has_bias)
    nc = _cache[has_bias]
    in_maps = [{**common, "x": shards[c]} for c in range(NCORES)]
    trace = bool(os.environ.get("KERNEL_TRACE"))
    last_exc = None
    for attempt in range(4):
        try:
            res = run_bass_kernel_spmd(nc, in_maps, core_ids=list(range(NCORES)),
                                       trace=trace)
            break
        except Exception as e:  # transient axon/device flakes — retry
            last_exc = e
    else:
        raise last_exc
    LAST_EXEC_NS = res.exec_time_ns
    LAST_TRACE = res.instructions_and_trace
    out = np.stack([res.results[c]["out"] for c in range(NCORES)])
    return out.reshape(B, NF, H, W).astype(np.float32)
